# revision 25
# baseline (speedup 1.0000x reference)
"""Trainium2 Bass kernel for nn_MultiHeadAttention_5360119185803.

Full-d_model attention (no head split) + residual + LayerNorm, B=4, T=S=2048,
E=1024, fp32 in/out.

Sharding: 8 cores; core c owns batch b=c//2 and query rows
[(c%2)*1024, (c%2+1)*1024). Each core projects the full key/value of its
batch (duplicated across the core pair; a pair AllGather measured slower).

Strategy: every matmul runs as an fp8e4 DoubleRow matmul (K=256 per
instruction, 2x bf16 PE rate). Host prep quantizes activations/weights to
fp8 and lays them out pre-transposed in the exact [128, pair-chunk, 2, cols]
SBUF layout DoubleRow wants, so the device does no transposes or dtype
converts on the critical path. All weights preload at t=0 on the scalar
HWDGE queue; xk streams block-major so P1 starts after ~0.75MB. Two PSUM
pools with cross-phase tag rings avoid pool-transition barriers. P6 (out
proj + LayerNorm) is interleaved into P5's second half, and the output is
written tile-major [p, tt, e] in tt-pairs (8KB descriptors) on alternating
queues, unpermuted on host.

Numerics (validated vs float64 reference on CPU; measured rms_rel 4.2e-3
vs the 2e-2 gate):
  - x, Wq/Wk/Wv/Wo in fp8e4; q/k/v psum f32, re-quantized to fp8 on
    eviction; ctx re-quantized to fp8 for the out-projection.
  - bk dropped (softmax-invariant); bv folded into the residual via
    bo' = bo + Wo@bv (attention rows sum to 1); bq added at q eviction.
  - scores stay unscaled in psum (sigma ~32); exp applies scale=1/32 and a
    fixed bias of -2 (cancels in the softmax ratio, keeps the fp8 expT in
    (0, ~30] well under the e4m3 max of 240).
  - rowsum via DoubleRow ones-matmul of the same fp8 expT (exact ratio);
    redistributed [1,T] -> [128, NT] through DRAM; reciprocal on DVE.
  - residual (query + bo') loaded bf16; y = psum/rowsum + res in bf16;
    LayerNorm stats in fp32 (bn_stats/bn_aggr on DVE), normalize on ACT.
"""

import sys

sys.path.insert(0, "/opt/trn_rl_repo")

import numpy as np
import ml_dtypes

import concourse.bacc as bacc
import concourse.tile as tile
from concourse import mybir
from concourse.bass_utils import run_bass_kernel_spmd

P = 128
E = 1024          # d_model
S = 2048          # kv seq len per batch
T = 1024          # query rows per core
NE = E // P       # 8 tiles of 128 along any d_model-like axis
EC2 = NE // 2     # 4 DoubleRow pair-chunks (K=256 each)
NST = S // P      # 16 s tiles
SC2 = NST // 2    # 8 DoubleRow pair-chunks along s
FD = 512          # psum free width (1 bank)

f32 = mybir.dt.float32
bf16 = mybir.dt.bfloat16
fp8 = mybir.dt.float8e4
AF = mybir.ActivationFunctionType
ALU = mybir.AluOpType
DR = mybir.MatmulPerfMode.DoubleRow

_cache = {}


def _build(apply_gb):
    nc = bacc.Bacc("TRN2", target_bir_lowering=False, debug=False, num_devices=8)

    # [p, (chunk, pair, col)] pre-transposed fp8 activations / weights
    xkt = nc.dram_tensor("xkt", [P, NE * S], fp8, kind="ExternalInput")
    xvt = nc.dram_tensor("xvt", [P, NE * S], fp8, kind="ExternalInput")
    xqt = nc.dram_tensor("xqt", [P, NE * T], fp8, kind="ExternalInput")
    wk8 = nc.dram_tensor("wk8", [P, NE * E], fp8, kind="ExternalInput")
    wv8 = nc.dram_tensor("wv8", [P, NE * E], fp8, kind="ExternalInput")
    wq8 = nc.dram_tensor("wq8", [P, NE * E], fp8, kind="ExternalInput")
    wo8 = nc.dram_tensor("wo8", [P, NE * E], fp8, kind="ExternalInput")
    bq2 = nc.dram_tensor("bq2", [P, NE], f32, kind="ExternalInput")
    xqr = nc.dram_tensor("xqr", [T, E], bf16, kind="ExternalInput")  # query + bo'
    if apply_gb:
        gam = nc.dram_tensor("gam", [E], f32, kind="ExternalInput")
        bet = nc.dram_tensor("bet", [E], f32, kind="ExternalInput")
    # out is [p, tt, e] tile-major: row t of the logical [T, E] output lives
    # at [t % 128, t // 128, :]; written in tt-pairs for 8KB-contiguous
    # descriptors, unpermuted on host.
    out = nc.dram_tensor("out", [P, NE, E], f32, kind="ExternalOutput")
    rs_dram = nc.dram_tensor("rs_scratch", [T], f32)

    with tile.TileContext(nc) as tc:
        consts = tc.alloc_tile_pool(name="consts", bufs=1, side="left")
        eps_t = consts.tile([P, 1], f32)
        nc.vector.memset(eps_t, 1e-6)
        nbias = consts.tile([P, 1], f32)
        nc.vector.memset(nbias, -2.0)
        ones_f = consts.tile([P, 2, 16], f32)
        nc.vector.memset(ones_f, 1.0)
        ones8 = consts.tile([P, 2, 16], fp8)
        nc.vector.tensor_copy(ones8, ones_f)
        recip_t = consts.tile([P, NE], f32)
        bq_sb = consts.tile([P, NE], f32)

        # all weights preloaded up-front on the scalar HWDGE queue into a
        # persistent pool (lazy gpsimd swdge loads cost 5-10us stalls each).
        # wk is split per pair-chunk so P1's first accumulation group can
        # start as soon as chunk 0 lands.
        wpool = tc.alloc_tile_pool(name="weights", bufs=1, side="left")
        wk8_v = wk8.ap().rearrange("p (c i e) -> p c i e", c=EC2, i=2)
        wk_cs = []
        for c in range(EC2):
            wkc = wpool.tile([P, 2, E], fp8, name=f"wkc{c}", tag=f"wkc{c}")
            nc.scalar.dma_start(out=wkc, in_=wk8_v[:, c, :, :])
            wk_cs.append(wkc)
        wv_sb = wpool.tile([P, EC2, 2, E], fp8)
        nc.scalar.dma_start(out=wv_sb, in_=wv8.ap())
        wq_sb = wpool.tile([P, EC2, 2, E], fp8)
        nc.scalar.dma_start(out=wq_sb, in_=wq8.ap())
        wo_sb = wpool.tile([P, EC2, 2, E], fp8)
        nc.scalar.dma_start(out=wo_sb, in_=wo8.ap())
        nc.scalar.dma_start(out=bq_sb, in_=bq2.ap())

        # two PSUM pools shared by all phases; cross-phase reuse rides the
        # per-tag ring dependencies instead of pool open/close barriers.
        # poolA: tags A0-A3 (4 banks). poolB: tags B0,B1 x bufs=2 (4 banks).
        poolA = tc.alloc_tile_pool(name="psA", bufs=1, space="PSUM")
        poolB = tc.alloc_tile_pool(name="psB", bufs=2, space="PSUM")
        _n = [0]

        def psA(i):
            _n[0] += 1
            return poolA.tile([P, FD], f32, name=f"A{i}_{_n[0]}", tag=f"A{i}")

        def psB(i):
            _n[0] += 1
            return poolB.tile([P, FD], f32, name=f"B{i}_{_n[0]}", tag=f"B{i}")

        # persistent activation/intermediate tiles (release order is LIFO
        # per side: left pops qT, kT after P4; right pops xk, xv, xq, then
        # expT, then v8)
        kT_pool = tc.alloc_tile_pool(name="kT", bufs=1, side="left")
        kT = kT_pool.tile([P, EC2, 2, S], fp8)     # [f, fc, fi, s] 2MB
        qT_pool = tc.alloc_tile_pool(name="qT", bufs=1, side="left")
        qT = qT_pool.tile([P, EC2, 2, T], fp8)     # [f, fc, fi, t] 1MB
        v_pool = tc.alloc_tile_pool(name="v8", bufs=1, side="right")
        v8 = v_pool.tile([P, SC2, 2, E], fp8)      # [s, sc, si, e'] 2MB

        xq_pool = tc.alloc_tile_pool(name="xq", bufs=1, side="right")
        xq = xq_pool.tile([P, EC2, 2, T], fp8)
        xv_pool = tc.alloc_tile_pool(name="xv", bufs=1, side="right")
        xv = xv_pool.tile([P, EC2, 2, S], fp8)
        xk_pool = tc.alloc_tile_pool(name="xk", bufs=1, side="right")
        xk = xk_pool.tile([P, EC2, 2, S], fp8)
        xkt_v = xkt.ap().rearrange("p (j c i s) -> p j c i s",
                                   j=S // FD, c=EC2, i=2)
        for j in range(S // FD):
            nc.sync.dma_start(out=xk[:, :, :, j * FD:(j + 1) * FD],
                              in_=xkt_v[:, j])
        nc.sync.dma_start(out=xv, in_=xvt.ap())
        nc.sync.dma_start(out=xq, in_=xqt.ap())

        # PE warm-up: dummy matmuls on const data while the first weight/
        # activation DMAs land, so the PE is at full clock for P1.
        wups = poolB.tile([P, FD], f32, name="warmup", tag="B0")
        ones8w = consts.tile([P, 2, FD], fp8)
        nc.vector.memset(ones8w, 1.0)
        NWU = 24
        for i in range(NWU):
            nc.tensor.matmul(wups[0:1, :], ones8[:, :, 0:1], ones8w,
                             start=(i == 0), stop=(i == NWU - 1), perf_mode=DR)
        wu_sb = consts.tile([P, 16], f32)
        nc.vector.tensor_copy(wu_sb[0:1, :], wups[0:1, 0:16])
        nc.sync.dma_start(out=rs_dram.ap()[0:16], in_=wu_sb[0:1, :])


        # ---- P1: kT[f, s] = sum_e WkT[e, f] * xkT[e, s] ----
        # sb outer: the first psum group only needs xk block 0 + wk chunk 0,
        # so the PE starts as soon as ~0.75MB of input has landed.
        for sb in range(S // FD):
            for fg in range(2):
                pss = [psA(fi) for fi in range(EC2)]
                for c in range(EC2):
                    for fi in range(EC2):
                        ft = fg * EC2 + fi
                        nc.tensor.matmul(
                            pss[fi], wk_cs[c][:, :, ft * P:(ft + 1) * P],
                            xk[:, c, :, sb * FD:(sb + 1) * FD],
                            start=(c == 0), stop=(c == EC2 - 1),
                            perf_mode=DR)
                for fi in range(EC2):
                    ft = fg * EC2 + fi
                    nc.vector.tensor_copy(
                        kT[:, ft // 2, ft % 2, sb * FD:(sb + 1) * FD],
                        pss[fi])
        xk_pool.release()

        # ---- P2: v[s, e'] = sum_e xvT[e, s] * WvT[e, e'] ----
        for ss in range(NST):
            pss = [psA(eh) for eh in range(E // FD)]
            for c in range(EC2):
                for eh in range(E // FD):
                    nc.tensor.matmul(
                        pss[eh], xv[:, c, :, ss * P:(ss + 1) * P],
                        wv_sb[:, c, :, eh * FD:(eh + 1) * FD],
                        start=(c == 0), stop=(c == EC2 - 1), perf_mode=DR)
            for eh in range(E // FD):
                nc.vector.tensor_copy(
                    v8[:, ss // 2, ss % 2, eh * FD:(eh + 1) * FD], pss[eh])
        xv_pool.release()

        # ---- P3: qT[f, t] = sum_e WqT[e, f] * xqT[e, t]; + bq on eviction ----
        for ft in range(NE):
            pss = [psA(2 + tb) for tb in range(T // FD)]
            for c in range(EC2):
                for tb in range(T // FD):
                    nc.tensor.matmul(
                        pss[tb], wq_sb[:, c, :, ft * P:(ft + 1) * P],
                        xq[:, c, :, tb * FD:(tb + 1) * FD],
                        start=(c == 0), stop=(c == EC2 - 1), perf_mode=DR)
            for tb in range(T // FD):
                # qT = psum + bq (per-partition bias on ACT, fp8 out)
                nc.scalar.activation(
                    qT[:, ft // 2, ft % 2, tb * FD:(tb + 1) * FD], pss[tb],
                    AF.Identity, bias=bq_sb[:, ft:ft + 1])
        xq_pool.release()

        # ---- P4: scoresT[s, t] -> expT fp8; rowsum (lagged to avoid stalls) ----
        expT_pool = tc.alloc_tile_pool(name="expT", bufs=1, side="right")
        expT = expT_pool.tile([P, SC2, 2, T], fp8)  # [s, sc, si, t] 2MB
        with tc.tile_pool(name="p4rw", bufs=1, side="right") as rwp:
            NTB = T // FD
            # rowsum accumulators live on poolA tags A0/A1 (idle during P4);
            # only row 0 of each tile is used.
            rpt = [psA(tb) for tb in range(NTB)]
            rps = [t[0:1, :] for t in rpt]

            def rowsum(sc):
                # sums expT over both si halves of chunk sc (contraction 256)
                for tb in range(NTB):
                    nc.tensor.matmul(
                        rps[tb], ones8[:, :, 0:1],
                        expT[:, sc, :, tb * FD:(tb + 1) * FD],
                        start=(sc == 0), stop=(sc == SC2 - 1), perf_mode=DR)

            for st in range(NST):
                pss = [psB(tb) for tb in range(NTB)]
                for c in range(EC2):
                    for tb in range(NTB):
                        nc.tensor.matmul(
                            pss[tb], kT[:, c, :, st * P:(st + 1) * P],
                            qT[:, c, :, tb * FD:(tb + 1) * FD],
                            start=(c == 0), stop=(c == EC2 - 1), perf_mode=DR)
                for tb in range(NTB):
                    nc.scalar.activation(
                        expT[:, st // 2, st % 2, tb * FD:(tb + 1) * FD],
                        pss[tb], AF.Exp, scale=1.0 / 32.0, bias=nbias)
                # chunk sc completes at st == 2*sc + 1; emit lagged by one
                # chunk so the PE never waits on the ACT exp.
                if st % 2 == 1 and st >= 3:
                    rowsum((st - 3) // 2)
            rowsum(SC2 - 1)
            rs_sb = rwp.tile([1, T], f32)
            for tb in range(NTB):
                nc.vector.tensor_copy(rs_sb[0:1, tb * FD:(tb + 1) * FD], rps[tb])
            nc.sync.dma_start(out=rs_dram.ap(), in_=rs_sb)
            rsT = rwp.tile([P, NE], f32)
            nc.sync.dma_start(out=rsT,
                              in_=rs_dram.ap().rearrange("(j p) -> p j", p=P))
            nc.vector.reciprocal(recip_t, rsT)
        qT_pool.release()
        kT_pool.release()

        # ---- P5 + P6 interleaved ----
        # P5 (ctxT[e', t] = sum_s v[s,e'] * expT[s,t]) runs tb-half outer, so
        # the P6 pipeline (out proj + LN + out DMA) for t-rows of tb=0 can
        # run while the PE crunches P5's tb=1, hiding the LN/write tail.
        ctxT_pool = tc.alloc_tile_pool(name="ctxT", bufs=1, side="left")
        ctxT = ctxT_pool.tile([P, EC2, 2, T], fp8)  # [e', ec, ei, t] 1MB
        with (
            tc.tile_pool(name="p6c", bufs=1, side="right") as p6c,
            tc.tile_pool(name="p6res", bufs=4, side="right") as resp,
            tc.tile_pool(name="p6y", bufs=3, side="right") as yp,
            tc.tile_pool(name="p6ln", bufs=4, side="right") as lnp,
            tc.tile_pool(name="p6out", bufs=3, side="right") as outp,
        ):
            if apply_gb:
                gam_sb = p6c.tile([P, E], f32)
                nc.gpsimd.dma_start(out=gam_sb, in_=gam.ap().partition_broadcast(P))
                bet_sb = p6c.tile([P, E], f32)
                nc.gpsimd.dma_start(out=bet_sb, in_=bet.ap().partition_broadcast(P))

            def p5_eg(tb, eg):
                pss = [psA(ei) for ei in range(4)]
                for c in range(SC2):
                    for ei in range(4):
                        e = eg * 4 + ei
                        nc.tensor.matmul(
                            pss[ei], v8[:, c, :, e * P:(e + 1) * P],
                            expT[:, c, :, tb * FD:(tb + 1) * FD],
                            start=(c == 0), stop=(c == SC2 - 1),
                            perf_mode=DR)
                for ei in range(4):
                    e = eg * 4 + ei
                    nc.scalar.activation(
                        ctxT[:, e // 2, e % 2, tb * FD:(tb + 1) * FD],
                        pss[ei], AF.Copy)

            opair = [None]

            def p6_tile(tt):
                dma_eng = nc.sync if (tt // 2) % 2 == 0 else nc.scalar
                res = resp.tile([P, E], bf16, name=f"res{tt}", tag="res")
                dma_eng.dma_start(out=res, in_=xqr.ap()[tt * P:(tt + 1) * P, :])
                y = yp.tile([P, E], bf16, name=f"y{tt}", tag="y")
                for gc in range(E // FD):
                    if tt < 4 or tt >= 6:
                        ps = psB(gc)
                    elif tt == 4:
                        ps = psA(gc)
                    else:
                        ps = psA(gc + 2)
                    for c in range(EC2):
                        nc.tensor.matmul(
                            ps, ctxT[:, c, :, tt * P:(tt + 1) * P],
                            wo_sb[:, c, :, gc * FD:(gc + 1) * FD],
                            start=(c == 0), stop=(c == EC2 - 1), perf_mode=DR)
                    nc.vector.scalar_tensor_tensor(
                        out=y[:, gc * FD:(gc + 1) * FD], in0=ps,
                        scalar=recip_t[:, tt:tt + 1],
                        in1=res[:, gc * FD:(gc + 1) * FD],
                        op0=ALU.mult, op1=ALU.add)
                stats = lnp.tile([P, 2, 6], f32, name=f"st{tt}", tag="st")
                nc.vector.bn_stats(stats[:, 0, :], y[:, 0:E // 2])
                nc.vector.bn_stats(stats[:, 1, :], y[:, E // 2:E])
                mv = lnp.tile([P, 2], f32, name=f"mv{tt}", tag="mv")
                nc.vector.bn_aggr(mv, stats)
                rstd = lnp.tile([P, 1], f32, name=f"rstd{tt}", tag="rstd")
                nc.scalar.activation(rstd, mv[:, 1:2], AF.Sqrt, bias=eps_t)
                nc.vector.reciprocal(rstd, rstd)
                nmr = lnp.tile([P, 1], f32, name=f"nmr{tt}", tag="nmr")
                nc.vector.tensor_scalar(out=nmr, in0=mv[:, 0:1], scalar1=rstd,
                                        scalar2=-1.0, op0=ALU.mult, op1=ALU.mult)
                if tt % 2 == 0:
                    opair[0] = outp.tile([P, 2, E], f32, name=f"o{tt}", tag="o")
                o = opair[0][:, tt % 2, :]
                # (y - mean) * rstd on ACT: Identity(y * rstd + (-mean*rstd))
                nc.scalar.activation(o, y, AF.Identity, bias=nmr,
                                     scale=rstd[:, 0:1])
                if apply_gb:
                    nc.vector.tensor_mul(o, o, gam_sb)
                    nc.vector.tensor_add(o, o, bet_sb)
                if tt >= NE - 2:
                    # last two tiles ship individually, each split across
                    # both queues, so the final write drains fastest
                    nc.sync.dma_start(
                        out=out.ap()[:, tt:tt + 1, 0:E // 2],
                        in_=opair[0][:, tt % 2:tt % 2 + 1, 0:E // 2])
                    nc.scalar.dma_start(
                        out=out.ap()[:, tt:tt + 1, E // 2:E],
                        in_=opair[0][:, tt % 2:tt % 2 + 1, E // 2:E])
                elif tt % 2 == 1:
                    dma_eng.dma_start(out=out.ap()[:, tt - 1:tt + 1, :],
                                      in_=opair[0])

            p5_eg(0, 0)
            p5_eg(0, 1)
            p5_eg(1, 0)
            p6_tile(0)
            p6_tile(1)
            p5_eg(1, 1)
            p6_tile(2)
            p6_tile(3)
            for tt in range(4, NE):
                p6_tile(tt)

        expT_pool.release()
        v_pool.release()
        ctxT_pool.release()
        poolB.release()
        poolA.release()
        wpool.release()
        consts.release()

    nc.compile()
    return nc


def _to_pair_layout(xT):
    """[E, cols] fp8 -> [128, NE*cols] with (chunk, pair, col) free layout."""
    cols = xT.shape[1]
    return np.ascontiguousarray(
        xT.reshape(EC2, 2, P, cols).transpose(2, 0, 1, 3).reshape(P, NE * cols))


def kernel(query, key, value, Wq, bq, Wk, bk, Wv, bv, Wo, bo, gamma, beta):
    query = np.asarray(query, dtype=np.float32)
    key = np.asarray(key, dtype=np.float32)
    value = np.asarray(value, dtype=np.float32)
    Wq = np.asarray(Wq, dtype=np.float32)
    bq = np.asarray(bq, dtype=np.float32)
    Wk = np.asarray(Wk, dtype=np.float32)
    Wv = np.asarray(Wv, dtype=np.float32)
    bv = np.asarray(bv, dtype=np.float32)
    Wo = np.asarray(Wo, dtype=np.float32)
    bo = np.asarray(bo, dtype=np.float32)
    gamma = np.asarray(gamma, dtype=np.float32)
    beta = np.asarray(beta, dtype=np.float32)

    f8 = ml_dtypes.float8_e4m3
    wk8 = _to_pair_layout(Wk.T.astype(f8))       # [e, f] pair layout
    wv8 = _to_pair_layout(Wv.T.astype(f8))
    wq8 = _to_pair_layout(Wq.T.astype(f8))
    wo8 = _to_pair_layout(Wo.T.astype(f8))
    bq2 = np.ascontiguousarray(bq.reshape(NE, P).T)
    bo2 = (bo + Wo @ bv).astype(np.float32)
    qres = (query + bo2).astype(np.float32)
    apply_gb = not (np.all(gamma == 1.0) and np.all(beta == 0.0))

    if apply_gb not in _cache:
        _cache[apply_gb] = _build(apply_gb)
    nc = _cache[apply_gb]

    B = query.shape[0]

    def _block_major(xp):
        # [P, NE*S] pair layout -> [P, (sb, c, i, FD)] so each sb-block DMA
        # reads one contiguous 4KB run per partition
        x4 = xp.reshape(P, EC2, 2, S // FD, FD)
        return np.ascontiguousarray(
            x4.transpose(0, 3, 1, 2, 4).reshape(P, NE * S))

    kt_b = [_block_major(_to_pair_layout(key[b].T.astype(f8)))
            for b in range(B)]
    vt_b = [_to_pair_layout(value[b].T.astype(f8)) for b in range(B)]

    in_maps = []
    for c in range(8):
        b, h = c // 2, c % 2
        m = {
            "xkt": kt_b[b],
            "xvt": vt_b[b],
            "xqt": _to_pair_layout(query[b, h * T:(h + 1) * T].T.astype(f8)),
            "wk8": wk8, "wv8": wv8, "wq8": wq8, "wo8": wo8,
            "bq2": bq2,
            "xqr": np.ascontiguousarray(
                qres[b, h * T:(h + 1) * T].astype(ml_dtypes.bfloat16)),
        }
        if apply_gb:
            m["gam"] = gamma
            m["bet"] = beta
        in_maps.append(m)

    global _saved_in_maps
    _saved_in_maps = in_maps
    res = run_bass_kernel_spmd(nc, in_maps, core_ids=list(range(8)))
    full = np.empty((B, 2 * T, E), dtype=np.float32)
    for c in range(8):
        b, h = c // 2, c % 2
        tiled = res.results[c]["out"]          # [p, tt, e] tile-major
        full[b, h * T:(h + 1) * T] = np.asarray(tiled).transpose(1, 0, 2
                                                                 ).reshape(T, E)
    return full


# revision 26
# speedup vs baseline: 1.0058x; 1.0058x over previous
"""Trainium2 Bass kernel for nn_MultiHeadAttention_5360119185803.

Full-d_model attention (no head split) + residual + LayerNorm, B=4, T=S=2048,
E=1024, fp32 in/out.

Sharding: 8 cores; core c owns batch b=c//2 and query rows
[(c%2)*1024, (c%2+1)*1024). Each core projects the full key/value of its
batch (duplicated across the core pair; a pair AllGather measured slower).

Strategy: every matmul runs as an fp8e4 DoubleRow matmul (K=256 per
instruction, 2x bf16 PE rate). Host prep quantizes activations/weights to
fp8 and lays them out pre-transposed in the exact [128, pair-chunk, 2, cols]
SBUF layout DoubleRow wants, so the device does no transposes or dtype
converts on the critical path. All weights preload at t=0 on the scalar
HWDGE queue; xk streams block-major so P1 starts after ~0.75MB. Two PSUM
pools with cross-phase tag rings avoid pool-transition barriers. P6 (out
proj + LayerNorm) is interleaved into P5's second half, and the output is
written tile-major [p, tt, e] in tt-pairs (8KB descriptors) on alternating
queues, unpermuted on host.

Numerics (validated vs float64 reference on CPU; measured rms_rel 4.2e-3
vs the 2e-2 gate):
  - x, Wq/Wk/Wv/Wo in fp8e4; q/k/v psum f32, re-quantized to fp8 on
    eviction; ctx re-quantized to fp8 for the out-projection.
  - bk dropped (softmax-invariant); bv folded into the residual via
    bo' = bo + Wo@bv (attention rows sum to 1); bq added at q eviction.
  - scores stay unscaled in psum (sigma ~32); exp applies scale=1/32 and a
    fixed bias of -2 (cancels in the softmax ratio, keeps the fp8 expT in
    (0, ~30] well under the e4m3 max of 240).
  - rowsum via DoubleRow ones-matmul of the same fp8 expT (exact ratio);
    redistributed [1,T] -> [128, NT] through DRAM; reciprocal on DVE.
  - residual (query + bo') loaded bf16; y = psum/rowsum + res in bf16;
    LayerNorm stats in fp32 (bn_stats/bn_aggr on DVE), normalize on ACT.
"""

import sys

sys.path.insert(0, "/opt/trn_rl_repo")

import numpy as np
import ml_dtypes

import concourse.bacc as bacc
import concourse.tile as tile
from concourse import mybir
from concourse.bass_utils import run_bass_kernel_spmd

P = 128
E = 1024          # d_model
S = 2048          # kv seq len per batch
T = 1024          # query rows per core
NE = E // P       # 8 tiles of 128 along any d_model-like axis
EC2 = NE // 2     # 4 DoubleRow pair-chunks (K=256 each)
NST = S // P      # 16 s tiles
SC2 = NST // 2    # 8 DoubleRow pair-chunks along s
FD = 512          # psum free width (1 bank)

f32 = mybir.dt.float32
bf16 = mybir.dt.bfloat16
fp8 = mybir.dt.float8e4
AF = mybir.ActivationFunctionType
ALU = mybir.AluOpType
DR = mybir.MatmulPerfMode.DoubleRow

_cache = {}


def _build(apply_gb):
    nc = bacc.Bacc("TRN2", target_bir_lowering=False, debug=False, num_devices=8)

    # [p, (chunk, pair, col)] pre-transposed fp8 activations / weights
    xkt = nc.dram_tensor("xkt", [P, NE * S], fp8, kind="ExternalInput")
    xvt = nc.dram_tensor("xvt", [P, NE * S], fp8, kind="ExternalInput")
    xqt = nc.dram_tensor("xqt", [P, NE * T], fp8, kind="ExternalInput")
    wk8 = nc.dram_tensor("wk8", [P, NE * E], fp8, kind="ExternalInput")
    wv8 = nc.dram_tensor("wv8", [P, NE * E], fp8, kind="ExternalInput")
    wq8 = nc.dram_tensor("wq8", [P, NE * E], fp8, kind="ExternalInput")
    wo8 = nc.dram_tensor("wo8", [P, NE * E], fp8, kind="ExternalInput")
    bq2 = nc.dram_tensor("bq2", [P, NE], f32, kind="ExternalInput")
    xqr = nc.dram_tensor("xqr", [T, E], bf16, kind="ExternalInput")  # query + bo'
    if apply_gb:
        gam = nc.dram_tensor("gam", [E], f32, kind="ExternalInput")
        bet = nc.dram_tensor("bet", [E], f32, kind="ExternalInput")
    # out is [p, tt, e] tile-major: row t of the logical [T, E] output lives
    # at [t % 128, t // 128, :]; written in tt-pairs for 8KB-contiguous
    # descriptors, unpermuted on host.
    out = nc.dram_tensor("out", [P, NE, E], f32, kind="ExternalOutput")
    rs_dram = nc.dram_tensor("rs_scratch", [T], f32)

    with tile.TileContext(nc) as tc:
        consts = tc.alloc_tile_pool(name="consts", bufs=1, side="left")
        eps_t = consts.tile([P, 1], f32)
        nc.vector.memset(eps_t, 1e-6)
        nbias = consts.tile([P, 1], f32)
        nc.vector.memset(nbias, -2.0)
        ones_f = consts.tile([P, 2, 16], f32)
        nc.vector.memset(ones_f, 1.0)
        ones8 = consts.tile([P, 2, 16], fp8)
        nc.vector.tensor_copy(ones8, ones_f)
        recip_t = consts.tile([P, NE], f32)
        bq_sb = consts.tile([P, NE], f32)

        # all weights preloaded up-front on the scalar HWDGE queue into a
        # persistent pool (lazy gpsimd swdge loads cost 5-10us stalls each).
        # wk is split per pair-chunk so P1's first accumulation group can
        # start as soon as chunk 0 lands.
        wpool = tc.alloc_tile_pool(name="weights", bufs=1, side="left")
        wk8_v = wk8.ap().rearrange("p (c i e) -> p c i e", c=EC2, i=2)
        wk_cs = []
        for c in range(EC2):
            wkc = wpool.tile([P, 2, E], fp8, name=f"wkc{c}", tag=f"wkc{c}")
            nc.scalar.dma_start(out=wkc, in_=wk8_v[:, c, :, :])
            wk_cs.append(wkc)
        wv_sb = wpool.tile([P, EC2, 2, E], fp8)
        nc.scalar.dma_start(out=wv_sb, in_=wv8.ap())
        wq_sb = wpool.tile([P, EC2, 2, E], fp8)
        nc.scalar.dma_start(out=wq_sb, in_=wq8.ap())
        wo_sb = wpool.tile([P, EC2, 2, E], fp8)
        nc.scalar.dma_start(out=wo_sb, in_=wo8.ap())
        nc.scalar.dma_start(out=bq_sb, in_=bq2.ap())

        # two PSUM pools shared by all phases; cross-phase reuse rides the
        # per-tag ring dependencies instead of pool open/close barriers.
        # poolA: tags A0-A3 (4 banks). poolB: tags B0,B1 x bufs=2 (4 banks).
        poolA = tc.alloc_tile_pool(name="psA", bufs=1, space="PSUM")
        poolB = tc.alloc_tile_pool(name="psB", bufs=2, space="PSUM")
        _n = [0]

        def psA(i):
            _n[0] += 1
            return poolA.tile([P, FD], f32, name=f"A{i}_{_n[0]}", tag=f"A{i}")

        def psB(i):
            _n[0] += 1
            return poolB.tile([P, FD], f32, name=f"B{i}_{_n[0]}", tag=f"B{i}")

        # persistent activation/intermediate tiles (release order is LIFO
        # per side: left pops qT, kT after P4; right pops xk, xv, xq, then
        # expT, then v8)
        kT_pool = tc.alloc_tile_pool(name="kT", bufs=1, side="left")
        kT = kT_pool.tile([P, EC2, 2, S], fp8)     # [f, fc, fi, s] 2MB
        qT_pool = tc.alloc_tile_pool(name="qT", bufs=1, side="left")
        qT = qT_pool.tile([P, EC2, 2, T], fp8)     # [f, fc, fi, t] 1MB
        v_pool = tc.alloc_tile_pool(name="v8", bufs=1, side="right")
        v8 = v_pool.tile([P, SC2, 2, E], fp8)      # [s, sc, si, e'] 2MB

        xq_pool = tc.alloc_tile_pool(name="xq", bufs=1, side="right")
        xq = xq_pool.tile([P, EC2, 2, T], fp8)
        xv_pool = tc.alloc_tile_pool(name="xv", bufs=1, side="right")
        xv = xv_pool.tile([P, EC2, 2, S], fp8)
        xk_pool = tc.alloc_tile_pool(name="xk", bufs=1, side="right")
        xk = xk_pool.tile([P, EC2, 2, S], fp8)
        xkt_v = xkt.ap().rearrange("p (j c i s) -> p j c i s",
                                   j=S // FD, c=EC2, i=2)
        for j in range(S // FD):
            nc.sync.dma_start(out=xk[:, :, :, j * FD:(j + 1) * FD],
                              in_=xkt_v[:, j])
        nc.sync.dma_start(out=xv, in_=xvt.ap())
        nc.sync.dma_start(out=xq, in_=xqt.ap())

        # PE warm-up: dummy matmuls on const data while the first weight/
        # activation DMAs land, so the PE is at full clock for P1.
        wups = poolB.tile([P, FD], f32, name="warmup", tag="B0")
        ones8w = consts.tile([P, 2, FD], fp8)
        nc.vector.memset(ones8w, 1.0)
        NWU = 24
        for i in range(NWU):
            nc.tensor.matmul(wups[0:1, :], ones8[:, :, 0:1], ones8w,
                             start=(i == 0), stop=(i == NWU - 1), perf_mode=DR)
        wu_sb = consts.tile([P, 16], f32)
        nc.vector.tensor_copy(wu_sb[0:1, :], wups[0:1, 0:16])
        nc.sync.dma_start(out=rs_dram.ap()[0:16], in_=wu_sb[0:1, :])


        # ---- P1: kT[f, s] = sum_e WkT[e, f] * xkT[e, s] ----
        # sb outer: the first psum group only needs xk block 0 + wk chunk 0,
        # so the PE starts as soon as ~0.75MB of input has landed.
        for sb in range(S // FD):
            for fg in range(2):
                pss = [psA(fi) for fi in range(EC2)]
                for c in range(EC2):
                    for fi in range(EC2):
                        ft = fg * EC2 + fi
                        nc.tensor.matmul(
                            pss[fi], wk_cs[c][:, :, ft * P:(ft + 1) * P],
                            xk[:, c, :, sb * FD:(sb + 1) * FD],
                            start=(c == 0), stop=(c == EC2 - 1),
                            perf_mode=DR)
                for fi in range(EC2):
                    ft = fg * EC2 + fi
                    nc.vector.tensor_copy(
                        kT[:, ft // 2, ft % 2, sb * FD:(sb + 1) * FD],
                        pss[fi])
        xk_pool.release()

        # ---- P2: v[s, e'] = sum_e xvT[e, s] * WvT[e, e'] ----
        for ss in range(NST):
            pss = [psA(eh) for eh in range(E // FD)]
            for c in range(EC2):
                for eh in range(E // FD):
                    nc.tensor.matmul(
                        pss[eh], xv[:, c, :, ss * P:(ss + 1) * P],
                        wv_sb[:, c, :, eh * FD:(eh + 1) * FD],
                        start=(c == 0), stop=(c == EC2 - 1), perf_mode=DR)
            for eh in range(E // FD):
                nc.vector.tensor_copy(
                    v8[:, ss // 2, ss % 2, eh * FD:(eh + 1) * FD], pss[eh])
        xv_pool.release()

        # ---- P3: qT[f, t] = sum_e WqT[e, f] * xqT[e, t]; + bq on eviction ----
        for ft in range(NE):
            pss = [psA(2 + tb) for tb in range(T // FD)]
            for c in range(EC2):
                for tb in range(T // FD):
                    nc.tensor.matmul(
                        pss[tb], wq_sb[:, c, :, ft * P:(ft + 1) * P],
                        xq[:, c, :, tb * FD:(tb + 1) * FD],
                        start=(c == 0), stop=(c == EC2 - 1), perf_mode=DR)
            for tb in range(T // FD):
                # qT = psum + bq (per-partition bias on ACT, fp8 out)
                nc.scalar.activation(
                    qT[:, ft // 2, ft % 2, tb * FD:(tb + 1) * FD], pss[tb],
                    AF.Identity, bias=bq_sb[:, ft:ft + 1])
        xq_pool.release()

        # ---- P4: scoresT[s, t] -> expT fp8; rowsum (lagged to avoid stalls) ----
        expT_pool = tc.alloc_tile_pool(name="expT", bufs=1, side="right")
        expT = expT_pool.tile([P, SC2, 2, T], fp8)  # [s, sc, si, t] 2MB
        with tc.tile_pool(name="p4rw", bufs=1, side="right") as rwp:
            NTB = T // FD
            # rowsum accumulators live on poolA tags A0/A1 (idle during P4);
            # only row 0 of each tile is used.
            rpt = [psA(tb) for tb in range(NTB)]
            rps = [t[0:1, :] for t in rpt]

            def rowsum(sc):
                # sums expT over both si halves of chunk sc (contraction 256)
                for tb in range(NTB):
                    nc.tensor.matmul(
                        rps[tb], ones8[:, :, 0:1],
                        expT[:, sc, :, tb * FD:(tb + 1) * FD],
                        start=(sc == 0), stop=(sc == SC2 - 1), perf_mode=DR)

            for st in range(NST):
                pss = [psB(tb) for tb in range(NTB)]
                for c in range(EC2):
                    for tb in range(NTB):
                        nc.tensor.matmul(
                            pss[tb], kT[:, c, :, st * P:(st + 1) * P],
                            qT[:, c, :, tb * FD:(tb + 1) * FD],
                            start=(c == 0), stop=(c == EC2 - 1), perf_mode=DR)
                for tb in range(NTB):
                    nc.scalar.activation(
                        expT[:, st // 2, st % 2, tb * FD:(tb + 1) * FD],
                        pss[tb], AF.Exp, scale=1.0 / 32.0, bias=nbias)
                # chunk sc completes at st == 2*sc + 1; emit lagged by one
                # chunk so the PE never waits on the ACT exp.
                if st % 2 == 1 and st >= 3:
                    rowsum((st - 3) // 2)
            rowsum(SC2 - 1)
            rs_sb = rwp.tile([1, T], f32)
            for tb in range(NTB):
                nc.vector.tensor_copy(rs_sb[0:1, tb * FD:(tb + 1) * FD], rps[tb])
            nc.sync.dma_start(out=rs_dram.ap(), in_=rs_sb)
            rsT = rwp.tile([P, NE], f32)
            nc.sync.dma_start(out=rsT,
                              in_=rs_dram.ap().rearrange("(j p) -> p j", p=P))
            nc.vector.reciprocal(recip_t, rsT)
        qT_pool.release()
        kT_pool.release()

        # ---- P5 + P6 interleaved ----
        # P5 (ctxT[e', t] = sum_s v[s,e'] * expT[s,t]) runs tb-half outer, so
        # the P6 pipeline (out proj + LN + out DMA) for t-rows of tb=0 can
        # run while the PE crunches P5's tb=1, hiding the LN/write tail.
        ctxT_pool = tc.alloc_tile_pool(name="ctxT", bufs=1, side="left")
        ctxT = ctxT_pool.tile([P, EC2, 2, T], fp8)  # [e', ec, ei, t] 1MB
        with (
            tc.tile_pool(name="p6c", bufs=1, side="right") as p6c,
            tc.tile_pool(name="p6res", bufs=4, side="right") as resp,
            tc.tile_pool(name="p6y", bufs=3, side="right") as yp,
            tc.tile_pool(name="p6ln", bufs=4, side="right") as lnp,
            tc.tile_pool(name="p6out", bufs=3, side="right") as outp,
        ):
            if apply_gb:
                gam_sb = p6c.tile([P, E], f32)
                nc.gpsimd.dma_start(out=gam_sb, in_=gam.ap().partition_broadcast(P))
                bet_sb = p6c.tile([P, E], f32)
                nc.gpsimd.dma_start(out=bet_sb, in_=bet.ap().partition_broadcast(P))

            def p5_eg(tb, eg):
                pss = [psA(ei) for ei in range(4)]
                for c in range(SC2):
                    for ei in range(4):
                        e = eg * 4 + ei
                        nc.tensor.matmul(
                            pss[ei], v8[:, c, :, e * P:(e + 1) * P],
                            expT[:, c, :, tb * FD:(tb + 1) * FD],
                            start=(c == 0), stop=(c == SC2 - 1),
                            perf_mode=DR)
                for ei in range(4):
                    e = eg * 4 + ei
                    nc.scalar.activation(
                        ctxT[:, e // 2, e % 2, tb * FD:(tb + 1) * FD],
                        pss[ei], AF.Copy)

            opair = [None]

            def p6_tile(tt):
                dma_eng = nc.sync if (tt // 2) % 2 == 0 else nc.scalar
                res = resp.tile([P, E], bf16, name=f"res{tt}", tag="res")
                dma_eng.dma_start(out=res, in_=xqr.ap()[tt * P:(tt + 1) * P, :])
                y = yp.tile([P, E], bf16, name=f"y{tt}", tag="y")
                for gc in range(E // FD):
                    ps = psB(gc) if tt < 4 else psA(gc + 2 * (tt % 2))
                    for c in range(EC2):
                        nc.tensor.matmul(
                            ps, ctxT[:, c, :, tt * P:(tt + 1) * P],
                            wo_sb[:, c, :, gc * FD:(gc + 1) * FD],
                            start=(c == 0), stop=(c == EC2 - 1), perf_mode=DR)
                    nc.vector.scalar_tensor_tensor(
                        out=y[:, gc * FD:(gc + 1) * FD], in0=ps,
                        scalar=recip_t[:, tt:tt + 1],
                        in1=res[:, gc * FD:(gc + 1) * FD],
                        op0=ALU.mult, op1=ALU.add)
                stats = lnp.tile([P, 2, 6], f32, name=f"st{tt}", tag="st")
                nc.vector.bn_stats(stats[:, 0, :], y[:, 0:E // 2])
                nc.vector.bn_stats(stats[:, 1, :], y[:, E // 2:E])
                mv = lnp.tile([P, 2], f32, name=f"mv{tt}", tag="mv")
                nc.vector.bn_aggr(mv, stats)
                rstd = lnp.tile([P, 1], f32, name=f"rstd{tt}", tag="rstd")
                nc.scalar.activation(rstd, mv[:, 1:2], AF.Sqrt, bias=eps_t)
                nc.vector.reciprocal(rstd, rstd)
                nmr = lnp.tile([P, 1], f32, name=f"nmr{tt}", tag="nmr")
                nc.vector.tensor_scalar(out=nmr, in0=mv[:, 0:1], scalar1=rstd,
                                        scalar2=-1.0, op0=ALU.mult, op1=ALU.mult)
                if tt % 2 == 0:
                    opair[0] = outp.tile([P, 2, E], f32, name=f"o{tt}", tag="o")
                o = opair[0][:, tt % 2, :]
                # (y - mean) * rstd on ACT: Identity(y * rstd + (-mean*rstd))
                nc.scalar.activation(o, y, AF.Identity, bias=nmr,
                                     scale=rstd[:, 0:1])
                if apply_gb:
                    nc.vector.tensor_mul(o, o, gam_sb)
                    nc.vector.tensor_add(o, o, bet_sb)
                if tt >= NE - 2:
                    # last two tiles ship individually so the final write
                    # overlaps the last LN chain
                    (nc.sync if tt % 2 == 0 else nc.scalar).dma_start(
                        out=out.ap()[:, tt:tt + 1, :], in_=opair[0][:, tt % 2:tt % 2 + 1, :])
                elif tt % 2 == 1:
                    dma_eng.dma_start(out=out.ap()[:, tt - 1:tt + 1, :],
                                      in_=opair[0])

            p5_eg(0, 0)
            p5_eg(0, 1)
            p5_eg(1, 0)
            p6_tile(0)
            p6_tile(1)
            p5_eg(1, 1)
            p6_tile(2)
            p6_tile(3)
            for tt in range(4, NE):
                p6_tile(tt)

        expT_pool.release()
        v_pool.release()
        ctxT_pool.release()
        poolB.release()
        poolA.release()
        wpool.release()
        consts.release()

    nc.compile()
    return nc


def _to_pair_layout(xT):
    """[E, cols] fp8 -> [128, NE*cols] with (chunk, pair, col) free layout."""
    cols = xT.shape[1]
    return np.ascontiguousarray(
        xT.reshape(EC2, 2, P, cols).transpose(2, 0, 1, 3).reshape(P, NE * cols))


def kernel(query, key, value, Wq, bq, Wk, bk, Wv, bv, Wo, bo, gamma, beta):
    query = np.asarray(query, dtype=np.float32)
    key = np.asarray(key, dtype=np.float32)
    value = np.asarray(value, dtype=np.float32)
    Wq = np.asarray(Wq, dtype=np.float32)
    bq = np.asarray(bq, dtype=np.float32)
    Wk = np.asarray(Wk, dtype=np.float32)
    Wv = np.asarray(Wv, dtype=np.float32)
    bv = np.asarray(bv, dtype=np.float32)
    Wo = np.asarray(Wo, dtype=np.float32)
    bo = np.asarray(bo, dtype=np.float32)
    gamma = np.asarray(gamma, dtype=np.float32)
    beta = np.asarray(beta, dtype=np.float32)

    f8 = ml_dtypes.float8_e4m3
    wk8 = _to_pair_layout(Wk.T.astype(f8))       # [e, f] pair layout
    wv8 = _to_pair_layout(Wv.T.astype(f8))
    wq8 = _to_pair_layout(Wq.T.astype(f8))
    wo8 = _to_pair_layout(Wo.T.astype(f8))
    bq2 = np.ascontiguousarray(bq.reshape(NE, P).T)
    bo2 = (bo + Wo @ bv).astype(np.float32)
    qres = (query + bo2).astype(np.float32)
    apply_gb = not (np.all(gamma == 1.0) and np.all(beta == 0.0))

    if apply_gb not in _cache:
        _cache[apply_gb] = _build(apply_gb)
    nc = _cache[apply_gb]

    B = query.shape[0]

    def _block_major(xp):
        # [P, NE*S] pair layout -> [P, (sb, c, i, FD)] so each sb-block DMA
        # reads one contiguous 4KB run per partition
        x4 = xp.reshape(P, EC2, 2, S // FD, FD)
        return np.ascontiguousarray(
            x4.transpose(0, 3, 1, 2, 4).reshape(P, NE * S))

    kt_b = [_block_major(_to_pair_layout(key[b].T.astype(f8)))
            for b in range(B)]
    vt_b = [_to_pair_layout(value[b].T.astype(f8)) for b in range(B)]

    in_maps = []
    for c in range(8):
        b, h = c // 2, c % 2
        m = {
            "xkt": kt_b[b],
            "xvt": vt_b[b],
            "xqt": _to_pair_layout(query[b, h * T:(h + 1) * T].T.astype(f8)),
            "wk8": wk8, "wv8": wv8, "wq8": wq8, "wo8": wo8,
            "bq2": bq2,
            "xqr": np.ascontiguousarray(
                qres[b, h * T:(h + 1) * T].astype(ml_dtypes.bfloat16)),
        }
        if apply_gb:
            m["gam"] = gamma
            m["bet"] = beta
        in_maps.append(m)

    global _saved_in_maps
    _saved_in_maps = in_maps
    res = run_bass_kernel_spmd(nc, in_maps, core_ids=list(range(8)))
    full = np.empty((B, 2 * T, E), dtype=np.float32)
    for c in range(8):
        b, h = c // 2, c % 2
        tiled = res.results[c]["out"]          # [p, tt, e] tile-major
        full[b, h * T:(h + 1) * T] = np.asarray(tiled).transpose(1, 0, 2
                                                                 ).reshape(T, E)
    return full
